# revision 10
# baseline (speedup 1.0000x reference)
"""Trainium2 Bass kernel for the Comatch retrieval problem.

Math: the reference's log_softmax + gumbel_softmax(hard=True) + straight-through
pipeline is numerically equivalent (to fp rounding) to a hard argmax selection:

    z[q,d,m,n]  = sim[q,d,m,n]/TEMP + BIAS[d,n] + gumbel[q,d,m,n]
                  (BIAS = (1-d_mask)*NEG/TEMP; LSE of log_softmax is constant
                   along n so it never changes the argmax; softmax is monotone)
    idx         = argmax_n z
    scores[q,d] = sum_m q_mask[q,m] * sim[q,d,m,idx]

On device (per core, d-axis sharded 200 -> 8*25):
  - PE projects Q@W and D@W with h-major (host-pretransposed) operands,
    row norms via ones-matmul column sums, 1/sqrt via ACT Ln/Exp.
  - per (qm-chunk of 128, d): PE matmul gives t = sim/TEMP in PSUM;
    DVE tensor_tensor_reduce computes z = t + g' (g' = gumbel + BIAS,
    prefolded on host) and its row max v in ONE pass;
    DVE scalar_tensor_tensor computes (z >= v) * g' and its row sum s in ONE
    pass.  Selected sim/TEMP = v - s.
  - scores = (q_mask*TEMP)^T @ (v - s) via a tiny PE matmul per chunk.
"""

import math
import os
import sys

import numpy as np

for _p in ("/opt/trn_rl_repo", os.path.expanduser("~/.axon_site/_ro/trn_rl_repo")):
    if os.path.isdir(_p) and _p not in sys.path:
        sys.path.append(_p)

from contextlib import ExitStack

import concourse.bass as bass
import concourse.mybir as mybir
import concourse.tile as tile
from concourse.bass import ds, ts
from concourse import bacc
from concourse.bass_utils import run_bass_kernel_spmd

F32 = mybir.dt.float32
AF = mybir.ActivationFunctionType
OP = mybir.AluOpType

NQ, M, ND, N, H, DIM = 32, 32, 200, 160, 768, 128
TEMP = 0.1
NEG = -10000.0

NCORES = 8
DLOC = ND // NCORES          # 25 docs per core
QM = NQ * M                  # 1024 query tokens
P = 128                      # partitions
NCH = QM // P                # 8 qm chunks
KH = H // P                  # 6 contraction chunks
FD = DLOC * N                # 4000 doc tokens per core
G_SIZES = [3] * 8 + [1]      # d-groups per 480-col psum tile

_CACHE = {}


def _trace_kernel(nc, repeat=1):
    qt_d = nc.dram_tensor("qt", [H, QM], F32, kind="ExternalInput").ap()
    dt_d = nc.dram_tensor("dt", [H, FD], F32, kind="ExternalInput").ap()
    w_d = nc.dram_tensor("w", [H, DIM], F32, kind="ExternalInput").ap()
    gb_d = nc.dram_tensor("gb", [NCH, P, DLOC, N], F32, kind="ExternalInput").ap()
    qw_d = nc.dram_tensor("qw", [NCH, P, 4], F32, kind="ExternalInput").ap()
    id_d = nc.dram_tensor("ident", [P, P], F32, kind="ExternalInput").ap()
    out_d = nc.dram_tensor("scores", [NCH, 4, DLOC], F32, kind="ExternalOutput").ap()

    with tile.TileContext(nc) as tc, ExitStack() as ctx:
        const = ctx.enter_context(tc.tile_pool(name="const", bufs=1))
        gpool = ctx.enter_context(tc.tile_pool(name="gpool", bufs=2))
        zpool = ctx.enter_context(tc.tile_pool(name="zpool", bufs=4))
        vpool = ctx.enter_context(tc.tile_pool(name="vpool", bufs=2))
        opool = ctx.enter_context(tc.tile_pool(name="opool", bufs=2))
        pp_t = ctx.enter_context(tc.tile_pool(name="pp_t", bufs=3, space="PSUM"))
        pp_sc = ctx.enter_context(tc.tile_pool(name="pp_sc", bufs=2, space="PSUM"))

        ones_col = const.tile([P, 1], F32)
        nc.vector.memset(ones_col, 1.0)
        ones_row = const.tile([1, P], F32)
        nc.vector.memset(ones_row, 1.0)
        ln10 = const.tile([1, 1], F32)
        nc.vector.memset(ln10, float(math.log(1.0 / TEMP)))

        for _rep in range(repeat):
            w_sb = const.tile([P, KH, DIM], F32, tag="w")
            nc.sync.dma_start(out=w_sb, in_=w_d.rearrange("(k p) d -> p k d", p=P))
            qw_sb = const.tile([P, NCH, 4], F32, tag="qw")
            nc.sync.dma_start(out=qw_sb, in_=qw_d.rearrange("c p j -> p c j"))
            id_sb = const.tile([P, P], F32, tag="ident")
            nc.sync.dma_start(out=id_sb, in_=id_d)
            qp_sb = const.tile([P, QM], F32, tag="qp")    # (Qp/TEMP)^T [dim, qm]
            dpt_sb = const.tile([P, FD], F32, tag="dpt")  # Dp^T [dim, d*n]

            # ---- Q projection + l2norm (scaled by 1/TEMP) ---------------
            with tc.tile_pool(name="projq", bufs=2) as projq, \
                 tc.tile_pool(name="ppq", bufs=1, space="PSUM") as ppq, \
                 tc.tile_pool(name="ppq1", bufs=1, space="PSUM") as ppq1:
                qt_sb = projq.tile([P, KH, QM], F32, tag="qt")
                nc.sync.dma_start(out=qt_sb,
                                  in_=qt_d.rearrange("(k p) m -> p k m", p=P))
                for h in range(2):
                    pq = ppq.tile([P, 512], F32, tag="pq")
                    for k in range(KH):
                        nc.tensor.matmul(pq, lhsT=w_sb[:, k, :],
                                         rhs=qt_sb[:, k, ts(h, 512)],
                                         start=(k == 0), stop=(k == KH - 1))
                    sq = projq.tile([P, 512], F32, tag="sq")
                    nc.scalar.activation(sq, pq, AF.Square)
                    n2p = ppq1.tile([1, 512], F32, tag="n2q")
                    nc.tensor.matmul(n2p, lhsT=ones_col, rhs=sq, start=True, stop=True)
                    lnq = projq.tile([1, 512], F32, tag="lnq")
                    nc.scalar.activation(lnq, n2p, AF.Ln)
                    r10 = projq.tile([1, 512], F32, tag="r10")
                    nc.scalar.activation(r10, lnq, AF.Exp, scale=-0.5, bias=ln10[:, :])
                    rbq = ppq1.tile([P, 512], F32, tag="rbq")
                    nc.tensor.matmul(rbq, lhsT=ones_row, rhs=r10, start=True, stop=True)
                    rbq_sb = projq.tile([P, 512], F32, tag="rbqsb")
                    nc.scalar.copy(rbq_sb, rbq)
                    nc.vector.tensor_mul(qp_sb[:, ts(h, 512)], pq, rbq_sb)

            # ---- D projection + l2norm ----------------------------------
            with tc.tile_pool(name="projd", bufs=2) as projd, \
                 tc.tile_pool(name="ppd", bufs=1, space="PSUM") as ppd, \
                 tc.tile_pool(name="ppd1", bufs=1, space="PSUM") as ppd1:
                dt_r = dt_d.rearrange("(k p) n -> p k n", p=P)
                for q4 in range(4):
                    dtq = projd.tile([P, KH, 1000], F32, tag="dtq")
                    nc.sync.dma_start(out=dtq, in_=dt_r[:, :, ds(q4 * 1000, 1000)])
                    for j in range(2):
                        off = q4 * 1000 + j * 500
                        pd = ppd.tile([P, 500], F32, tag="pd")
                        for k in range(KH):
                            nc.tensor.matmul(pd, lhsT=w_sb[:, k, :],
                                             rhs=dtq[:, k, ds(j * 500, 500)],
                                             start=(k == 0), stop=(k == KH - 1))
                        sqd = projd.tile([P, 500], F32, tag="sqd")
                        nc.scalar.activation(sqd, pd, AF.Square)
                        n2d = ppd1.tile([1, 500], F32, tag="n2d")
                        nc.tensor.matmul(n2d, lhsT=ones_col, rhs=sqd,
                                         start=True, stop=True)
                        lnd = projd.tile([1, 500], F32, tag="lnd")
                        nc.scalar.activation(lnd, n2d, AF.Ln)
                        rd = projd.tile([1, 500], F32, tag="rd")
                        nc.scalar.activation(rd, lnd, AF.Exp, scale=-0.5)
                        rbd = ppd1.tile([P, 500], F32, tag="rbd")
                        nc.tensor.matmul(rbd, lhsT=ones_row, rhs=rd,
                                         start=True, stop=True)
                        rbd_sb = projd.tile([P, 500], F32, tag="rbdsb")
                        nc.scalar.copy(rbd_sb, rbd)
                        nc.vector.tensor_mul(dpt_sb[:, ds(off, 500)], pd, rbd_sb)

            # ---- main phase: z = sim/TEMP + g' on PE, max + select on DVE --
            for c in range(NCH):
                gbt = gpool.tile([P, DLOC, N], F32, tag="gb")
                nc.sync.dma_start(out=gbt, in_=gb_d[c])
                zsb = gpool.tile([P, DLOC, N], F32, tag="zsb")
                for sb0 in range(0, len(G_SIZES), 3):
                    blk = G_SIZES[sb0:sb0 + 3]
                    tps = []
                    for j, gsz in enumerate(blk):
                        d0 = sum(G_SIZES[:sb0 + j])
                        tp = pp_t.tile([P, G_SIZES[0] * N], F32, tag="t")
                        nc.tensor.matmul(tp[:, : gsz * N], lhsT=qp_sb[:, ts(c, P)],
                                         rhs=dpt_sb[:, ds(d0 * N, gsz * N)],
                                         start=True, stop=False)
                        tps.append(tp)
                    for j, gsz in enumerate(blk):
                        d0 = sum(G_SIZES[:sb0 + j])
                        nc.tensor.matmul(tps[j][:, : gsz * N], lhsT=id_sb,
                                         rhs=gbt[:, ds(d0, gsz), :],
                                         start=False, stop=True)
                    for j, gsz in enumerate(blk):
                        d0 = sum(G_SIZES[:sb0 + j])
                        nc.scalar.copy(zsb[:, ds(d0, gsz), :], tps[j][:, : gsz * N])
                v = vpool.tile([P, DLOC], F32, tag="v")
                nc.vector.tensor_reduce(out=v, in_=zsb, axis=mybir.AxisListType.X,
                                        op=OP.max)
                s = vpool.tile([P, DLOC], F32, tag="s")
                for d in range(DLOC):
                    wt = zpool.tile([P, N], F32, tag="w")
                    nc.vector.scalar_tensor_tensor(
                        out=wt, in0=zsb[:, d, :], scalar=v[:, ds(d, 1)],
                        in1=gbt[:, d, :],
                        op0=OP.is_ge, op1=OP.mult, accum_out=s[:, ds(d, 1)])
                sel = vpool.tile([P, DLOC], F32, tag="sel")
                nc.vector.tensor_sub(sel, v, s)
                scp = pp_sc.tile([4, DLOC], F32, tag="sc")
                nc.tensor.matmul(scp, lhsT=qw_sb[:, c, :], rhs=sel, start=True,
                                 stop=True)
                sco = opool.tile([4, DLOC], F32, tag="sco")
                nc.scalar.copy(sco, scp)
                nc.sync.dma_start(out=out_d[c], in_=sco)
    return nc


def _get_nc(repeat=1):
    key = ("nc", repeat)
    if key not in _CACHE:
        nc = bacc.Bacc("TRN2", target_bir_lowering=False, debug=False)
        _trace_kernel(nc, repeat=repeat)
        nc.compile()
        _CACHE[key] = nc
    return _CACHE[key]


def _prep_inputs(Q, D, q_mask, d_mask, gumbel_noise, W_proj):
    Q = np.ascontiguousarray(Q, dtype=np.float32)
    D = np.ascontiguousarray(D, dtype=np.float32)
    q_mask = np.asarray(q_mask, dtype=np.float32)
    d_mask = np.asarray(d_mask, dtype=np.float32)
    W = np.ascontiguousarray(W_proj, dtype=np.float32)

    Qt = np.ascontiguousarray(Q.reshape(QM, H).T)                 # [768, 1024]
    qwf = q_mask * np.float32(TEMP)                               # fold 1/TEMP back out
    qw = np.zeros((NCH, P, 4), np.float32)
    for c in range(NCH):
        for j in range(4):
            qw[c, j * 32:(j + 1) * 32, j] = qwf[4 * c + j]

    ident = np.eye(P, dtype=np.float32)
    in_maps = []
    for k in range(NCORES):
        dsl = slice(k * DLOC, (k + 1) * DLOC)
        Dt = np.ascontiguousarray(D[dsl].reshape(DLOC * N, H).T)  # [768, 4000]
        bias = (1.0 - d_mask[dsl]) * np.float32(NEG / TEMP)       # [25, 160]
        gb = gumbel_noise[:, dsl].astype(np.float32).transpose(0, 2, 1, 3)
        gb = gb + bias[None, None, :, :]                          # [q, m, d, n]
        gb = np.ascontiguousarray(gb.reshape(NCH, P, DLOC, N))
        in_maps.append({"qt": Qt, "dt": Dt, "w": W, "gb": gb, "qw": qw,
                        "ident": ident})
    return in_maps


def _postprocess(results):
    full = np.empty((NQ, ND), np.float32)
    for k, res in enumerate(results):
        full[:, k * DLOC:(k + 1) * DLOC] = res["scores"].reshape(NQ, DLOC)
    return full


def kernel(Q, D, q_mask, d_mask, gumbel_noise, W_proj):
    nc = _get_nc()
    in_maps = _prep_inputs(Q, D, q_mask, d_mask, gumbel_noise, W_proj)
    res = run_bass_kernel_spmd(nc, in_maps, core_ids=list(range(NCORES)))
    return _postprocess(res.results)


def run_repeat(inputs, repeat=1):
    """Run the program with the body repeated `repeat` times; returns scores."""
    nc = _get_nc(repeat)
    in_maps = _prep_inputs(**inputs)
    res = run_bass_kernel_spmd(nc, in_maps, core_ids=list(range(NCORES)))
    return _postprocess(res.results)
